# revision 13
# baseline (speedup 1.0000x reference)
"""Low-pass FFT filtering kernel for Trainium2 (8 NeuronCores).

Math: reference does, per (batch b, channel i), with X = x[b,:,:,i] (256x256):
    out_i = irfft(rfft(X, axis=0) * mask) + irfft(rfft(X, axis=1) * mask)
with mask keeping rfft modes 0..15 (ortho norm). That filter is an orthogonal
projection P = W @ W.T where W [256, 31] is the orthonormal basis
{1/sqrt(n), sqrt(2/n)cos(2pi k t/n), -sqrt(2/n)sin(2pi k t/n)}_{k=1..15}.
So  out_i = P @ X_i + X_i @ P = W @ (W.T @ X_i) + (X_i @ W) @ W.T.

Device schedule (per core = one batch, channel-major layouts):
  C = W.T @ Xcm   [31, I*N]   (Xcm = x[b] as [m, (i, n)])
  D = W.T @ Xt    [31, I*M]   (Xt  = x[b] as [n, (i, m)], host-transposed)
  out[m-tile, n'] per (i, j):  single K=63 matmul with
     lhsT = [Wt_j ; 0 ; D_i,j]  (63 x 128),  rhs = [C_i ; 0 ; Wt] (63 x 256)
  which accumulates both terms in one PSUM pass.
Inputs/weights are fp16 on device (PE runs fp16 at full rate vs 4x-cost
fp32 LOW_HIGH mode); accumulation is fp32 in PSUM; the output is staged fp16
on device and upcast to fp32 on host (rel err ~7e-4 end to end).
Sharding: batch b -> core b (8 cores, no communication).
"""

import os
import sys
import types

import numpy as np

import concourse.bass as bass
import concourse.bacc as bacc
import concourse.tile as tile
from concourse import mybir
from concourse.bass_utils import run_bass_kernel_spmd

B, M, N, I = 8, 256, 256, 32
KMAX = 16           # modes kept: 0..15
R = 2 * KMAX - 1    # 31 real basis vectors
FREE = I * N        # 8192
UW = 1024           # unit width (cols) = 4 channels
NU = FREE // UW     # 8 units
F32 = mybir.dt.float32
F16 = mybir.dt.float16
NPDT = np.float16

LAST_RESULTS = None  # BassKernelResults of the most recent run (for test.py)


def _ensure_ntff_hook():
    """Provide antenv.axon_hooks if the image lacks it, so trace=True works."""
    try:
        from antenv.axon_hooks import get_axon_ntff_profile_hook  # noqa: F401
        return
    except ImportError:
        pass
    try:
        from trn_agent_boot.trn_boot import _ntff_profile_via_ctypes
        hook = _ntff_profile_via_ctypes("/opt/axon/libaxon_pjrt.so")
    except Exception:
        hook = None
    mod = types.ModuleType("antenv.axon_hooks")
    _state = {"hook": hook}
    mod.get_axon_ntff_profile_hook = lambda: _state["hook"]
    mod.set_axon_ntff_profile_hook = lambda h: _state.update(hook=h)
    sys.modules["antenv.axon_hooks"] = mod
    try:
        import antenv
        antenv.axon_hooks = mod
    except ImportError:
        pass


def _basis():
    t = np.arange(N)
    cols = [np.ones(N) / np.sqrt(N)]
    for k in range(1, KMAX):
        cols.append(np.sqrt(2.0 / N) * np.cos(2 * np.pi * k * t / N))
        cols.append(-np.sqrt(2.0 / N) * np.sin(2 * np.pi * k * t / N))
    return np.stack(cols, axis=1).astype(np.float32)  # [256, 31]


def _build_nc():
    nc = bacc.Bacc("TRN2", target_bir_lowering=False, debug=False,
                   enable_asserts=False, num_devices=8)

    xc = nc.declare_dram_parameter("xc", [M, FREE], F16, isOutput=False)
    xt = nc.declare_dram_parameter("xt", [N, I * M], F16, isOutput=False)
    w2 = nc.declare_dram_parameter("w2", [128, 2 * R], F16, isOutput=False)
    wz = nc.declare_dram_parameter("wz", [R + 1, UW], F16, isOutput=False)
    zw = nc.declare_dram_parameter("zw", [R + 1, UW], F16, isOutput=False)
    out = nc.declare_dram_parameter("out", [M, FREE], F16, isOutput=True)

    with tile.TileContext(nc) as tc:
        with (
            tc.tile_pool(name="const", bufs=1) as constp,
            tc.tile_pool(name="xin", bufs=5) as xin,
            tc.tile_pool(name="oput", bufs=2) as outp,
            tc.tile_pool(name="pcd", bufs=2, space=bass.MemorySpace.PSUM) as pcdp,
            tc.tile_pool(name="p2", bufs=4, space=bass.MemorySpace.PSUM) as p2p,
        ):
            w2sb = constp.tile([128, 2 * R], F16)
            nc.sync.dma_start(out=w2sb[:], in_=w2[:])

            # persistent ping-pong staging tiles; const rows loaded ONCE:
            #   LG rows 0..30 = tiled W^T (lhsT const), row 31 = 0,
            #      rows 32..62 <- D per unit
            #   RG rows 0..30 <- C per unit, row 31 = 0,
            #      rows 32..62 = tiled W^T (rhs const)
            LGs = [constp.tile([63, UW], F16, tag=f"lg{k}", name=f"lg{k}")
                   for k in range(2)]
            RGs = [constp.tile([63, UW], F16, tag=f"rg{k}", name=f"rg{k}")
                   for k in range(2)]
            for k in range(2):
                nc.gpsimd.dma_start(out=LGs[k][0:32, :], in_=wz[:])
                nc.gpsimd.dma_start(out=RGs[k][31:63, :], in_=zw[:])

            units = [None] * NU

            def emit_dma(u):
                c0 = u * UW
                x0 = xin.tile([128, UW], F16, tag="x0")
                x1 = xin.tile([128, UW], F16, tag="x1")
                t0 = xin.tile([128, UW], F16, tag="t0")
                t1 = xin.tile([128, UW], F16, tag="t1")
                # 512-col pieces so the first matmuls start after only 128KB
                # has landed and per-piece deps release early. Steady state
                # uses 2 rings (x on SP, xt on ACT) whose combined descriptor
                # rate covers the ~332GB/s DMA ceiling; the first two units
                # fan out across 4 rings so the DMA engines saturate
                # immediately instead of ramping.
                for p in range(UW // 512):
                    ps = slice(p * 512, (p + 1) * 512)
                    gs = slice(c0 + p * 512, c0 + (p + 1) * 512)
                    nc.sync.dma_start(out=x0[:, ps], in_=xc[0:128, gs])
                    nc.sync.dma_start(out=x1[:, ps], in_=xc[128:256, gs])
                    nc.scalar.dma_start(out=t0[:, ps], in_=xt[0:128, gs])
                    nc.scalar.dma_start(out=t1[:, ps], in_=xt[128:256, gs])
                units[u] = (x0, x1, t0, t1)

            p1state = [None] * NU
            p2state = [None] * NU

            def emit_p1_f(u, f):
                x0, x1, t0, t1 = units[u]
                if f == 0:
                    p1state[u] = pcdp.tile([63, UW], F32, tag="pcd", name=f"pcd{u}")
                pcd = p1state[u]
                fsl = slice(f * 512, (f + 1) * 512)
                nc.tensor.matmul(pcd[0:R, fsl], w2sb[:, 0:R], x0[:, fsl],
                                 start=True, stop=False)
                nc.tensor.matmul(pcd[0:R, fsl], w2sb[:, R:2 * R],
                                 x1[:, fsl], start=False, stop=True)
                nc.tensor.matmul(pcd[32:63, fsl], w2sb[:, 0:R], t0[:, fsl],
                                 start=True, stop=False)
                nc.tensor.matmul(pcd[32:63, fsl], w2sb[:, R:2 * R],
                                 t1[:, fsl], start=False, stop=True)

            def emit_copies(u):
                pcd = p1state[u]
                LG, RG = LGs[u % 2], RGs[u % 2]
                # one wide copy per operand: C on ACT, D on DVE
                nc.scalar.copy(RG[0:R, :], pcd[0:R, :])
                nc.vector.tensor_copy(LG[32:63, :], pcd[32:63, :])

            def emit_p2_half(u, j):
                LG, RG = LGs[u % 2], RGs[u % 2]
                c0 = u * UW
                if j == 0:
                    p2state[u] = (outp.tile([128, UW], F16, tag="o0", name=f"o0_{u}"),
                                  outp.tile([128, UW], F16, tag="o1", name=f"o1_{u}"))
                oj = p2state[u][j]
                for pr in range(2):          # channel pairs
                    p2 = p2p.tile([128, 2 * N], F32, tag="p2")
                    for s in range(2):
                        il = 2 * pr + s
                        csl = slice(il * N, (il + 1) * N)
                        jsl = slice(il * N + j * 128, il * N + (j + 1) * 128)
                        nc.tensor.matmul(p2[:, s * N:(s + 1) * N],
                                         LG[:, jsl], RG[:, csl],
                                         start=True, stop=True)
                    osl = slice(2 * pr * N, (2 * pr + 2) * N)
                    # split casts across ACT/DVE to balance engine load
                    if (j + pr) % 2 == 0:
                        nc.vector.tensor_copy(oj[:, osl], p2[:])
                    else:
                        nc.scalar.copy(oj[:, osl], p2[:])
                # outputs drain on the GpSimd ring as produced (inputs
                # keep the SP/ACT rings), in 512-col pieces. The last two
                # units ride the SP/ACT rings instead - input traffic is
                # done by then and GpSimd's expensive dge drain overlaps
                # the remaining compute instead of extending the tail.
                ring = (nc.gpsimd if u < NU - 2
                        else (nc.sync if j == 0 else nc.scalar))
                for p in range(UW // 512):
                    ps = slice(p * 512, (p + 1) * 512)
                    gs = slice(c0 + p * 512, c0 + (p + 1) * 512)
                    ring.dma_start(out=out[j * 128:(j + 1) * 128, gs],
                                   in_=oj[:, ps])

            # software pipeline, interleaved at f-block granularity so the
            # PE queue always has ready P2 work from unit u-1 between P1
            # groups of unit u that may stall on input DMA.
            emit_dma(0)
            emit_dma(1)
            emit_dma(2)
            emit_p1_f(0, 0)
            emit_p1_f(0, 1)
            emit_copies(0)
            for u in range(1, NU):
                if u + 2 < NU:
                    emit_dma(u + 2)
                emit_p1_f(u, 0)
                emit_p2_half(u - 1, 0)
                emit_p1_f(u, 1)
                emit_p2_half(u - 1, 1)
                emit_copies(u)
            emit_p2_half(NU - 1, 0)
            emit_p2_half(NU - 1, 1)

    nc.finalize()
    return nc


_NC = None


def kernel(x: np.ndarray) -> np.ndarray:
    global _NC, LAST_RESULTS
    x = np.asarray(x)
    assert x.shape == (B, M, N, I), x.shape

    W = _basis().astype(NPDT)          # [256, 31]
    Wt = W.T.copy()                    # [31, 256]
    w2_np = np.concatenate([W[0:128, :], W[128:256, :]], axis=1)  # [128, 62]
    wtile = np.tile(Wt, (1, UW // N))                             # [31, 1024]
    wz_np = np.concatenate([wtile, np.zeros((1, UW), NPDT)], axis=0)
    zw_np = np.concatenate([np.zeros((1, UW), NPDT), wtile], axis=0)

    if _NC is None:
        _NC = _build_nc()

    xq = np.asarray(x, dtype=NPDT)
    in_maps = []
    for b in range(B):
        xcm = np.ascontiguousarray(xq[b].transpose(0, 2, 1)).reshape(M, FREE)
        xtm = np.ascontiguousarray(xq[b].transpose(1, 2, 0)).reshape(N, I * M)
        in_maps.append({
            "xc": xcm, "xt": xtm,
            "w2": w2_np, "wz": wz_np, "zw": zw_np,
        })

    trace = bool(int(os.environ.get("KERNEL_TRACE", "0")))
    if trace:
        _ensure_ntff_hook()
    last_err = None
    for attempt in range(3):
        try:
            LAST_RESULTS = run_bass_kernel_spmd(_NC, in_maps, list(range(B)),
                                                trace=trace and attempt == 0)
            break
        except Exception as e:  # rare transient NRT_EXEC_UNIT_UNRECOVERABLE
            last_err = e
            import time as _time
            _time.sleep(2.0)
            try:
                import jax
                jax.clear_caches()
                jax.extend.backend.clear_backends()
            except Exception:
                pass
    else:
        raise last_err

    out = np.empty((B, M, N, I), np.float32)
    for b in range(B):
        dev = LAST_RESULTS.results[b]["out"].astype(np.float32).reshape(M, I, N)
        out[b] = dev.transpose(0, 2, 1)
    return out



# revision 14
# speedup vs baseline: 1.2011x; 1.2011x over previous
"""Low-pass FFT filtering kernel for Trainium2 (8 NeuronCores).

Math: reference does, per (batch b, channel i), with X = x[b,:,:,i] (256x256):
    out_i = irfft(rfft(X, axis=0) * mask) + irfft(rfft(X, axis=1) * mask)
with mask keeping rfft modes 0..15 (ortho norm). That filter is an orthogonal
projection P = W @ W.T where W [256, 31] is the orthonormal basis
{1/sqrt(n), sqrt(2/n)cos(2pi k t/n), -sqrt(2/n)sin(2pi k t/n)}_{k=1..15}.
So  out_i = P @ X_i + X_i @ P = W @ (W.T @ X_i) + (X_i @ W) @ W.T.

Device schedule (per core = one batch, channel-major layouts):
  C = W.T @ Xcm   [31, I*N]   (Xcm = x[b] as [m, (i, n)])
  D = W.T @ Xt    [31, I*M]   (Xt  = x[b] as [n, (i, m)], host-transposed)
  out[m-tile, n'] per (i, j):  single K=63 matmul with
     lhsT = [Wt_j ; 0 ; D_i,j]  (63 x 128),  rhs = [C_i ; 0 ; Wt] (63 x 256)
  which accumulates both terms in one PSUM pass.
Inputs/weights are fp16 on device (PE runs fp16 at full rate vs 4x-cost
fp32 LOW_HIGH mode); accumulation is fp32 in PSUM; the output is staged fp16
on device and upcast to fp32 on host (rel err ~7e-4 end to end).
Sharding: batch b -> core b (8 cores, no communication).
"""

import os
import sys
import types

import numpy as np

import concourse.bass as bass
import concourse.bacc as bacc
import concourse.tile as tile
from concourse import mybir
from concourse.bass_utils import run_bass_kernel_spmd

B, M, N, I = 8, 256, 256, 32
KMAX = 16           # modes kept: 0..15
R = 2 * KMAX - 1    # 31 real basis vectors
FREE = I * N        # 8192
UW = 1024           # unit width (cols) = 4 channels
NU = FREE // UW     # 8 units
F32 = mybir.dt.float32
F16 = mybir.dt.float16
NPDT = np.float16

LAST_RESULTS = None  # BassKernelResults of the most recent run (for test.py)


def _ensure_ntff_hook():
    """Provide antenv.axon_hooks if the image lacks it, so trace=True works."""
    try:
        from antenv.axon_hooks import get_axon_ntff_profile_hook  # noqa: F401
        return
    except ImportError:
        pass
    try:
        from trn_agent_boot.trn_boot import _ntff_profile_via_ctypes
        hook = _ntff_profile_via_ctypes("/opt/axon/libaxon_pjrt.so")
    except Exception:
        hook = None
    mod = types.ModuleType("antenv.axon_hooks")
    _state = {"hook": hook}
    mod.get_axon_ntff_profile_hook = lambda: _state["hook"]
    mod.set_axon_ntff_profile_hook = lambda h: _state.update(hook=h)
    sys.modules["antenv.axon_hooks"] = mod
    try:
        import antenv
        antenv.axon_hooks = mod
    except ImportError:
        pass


def _basis():
    t = np.arange(N)
    cols = [np.ones(N) / np.sqrt(N)]
    for k in range(1, KMAX):
        cols.append(np.sqrt(2.0 / N) * np.cos(2 * np.pi * k * t / N))
        cols.append(-np.sqrt(2.0 / N) * np.sin(2 * np.pi * k * t / N))
    return np.stack(cols, axis=1).astype(np.float32)  # [256, 31]


def _build_nc():
    nc = bacc.Bacc("TRN2", target_bir_lowering=False, debug=False,
                   enable_asserts=False, num_devices=8)

    xc = nc.declare_dram_parameter("xc", [M, FREE], F16, isOutput=False)
    xt = nc.declare_dram_parameter("xt", [N, I * M], F16, isOutput=False)
    w2 = nc.declare_dram_parameter("w2", [128, 2 * R], F16, isOutput=False)
    wz = nc.declare_dram_parameter("wz", [R + 1, UW], F16, isOutput=False)
    zw = nc.declare_dram_parameter("zw", [R + 1, UW], F16, isOutput=False)
    out = nc.declare_dram_parameter("out", [M, FREE], F16, isOutput=True)

    with tile.TileContext(nc) as tc:
        with (
            tc.tile_pool(name="const", bufs=1) as constp,
            tc.tile_pool(name="xin", bufs=3) as xin,
            tc.tile_pool(name="oput", bufs=2) as outp,
            tc.tile_pool(name="pcd", bufs=2, space=bass.MemorySpace.PSUM) as pcdp,
            tc.tile_pool(name="p2", bufs=4, space=bass.MemorySpace.PSUM) as p2p,
        ):
            w2sb = constp.tile([128, 2 * R], F16)
            nc.sync.dma_start(out=w2sb[:], in_=w2[:])

            # persistent ping-pong staging tiles; const rows loaded ONCE:
            #   LG rows 0..30 = tiled W^T (lhsT const), row 31 = 0,
            #      rows 32..62 <- D per unit
            #   RG rows 0..30 <- C per unit, row 31 = 0,
            #      rows 32..62 = tiled W^T (rhs const)
            LGs = [constp.tile([63, UW], F16, tag=f"lg{k}", name=f"lg{k}")
                   for k in range(2)]
            RGs = [constp.tile([63, UW], F16, tag=f"rg{k}", name=f"rg{k}")
                   for k in range(2)]
            for k in range(2):
                nc.gpsimd.dma_start(out=LGs[k][0:32, :], in_=wz[:])
                nc.gpsimd.dma_start(out=RGs[k][31:63, :], in_=zw[:])

            units = [None] * NU

            def emit_dma(u):
                c0 = u * UW
                x0 = xin.tile([128, UW], F16, tag="x0")
                x1 = xin.tile([128, UW], F16, tag="x1")
                t0 = xin.tile([128, UW], F16, tag="t0")
                t1 = xin.tile([128, UW], F16, tag="t1")
                # 512-col pieces so the first matmuls start after only 128KB
                # has landed and per-piece deps release early. Steady state
                # uses 2 rings (x on SP, xt on ACT) whose combined descriptor
                # rate covers the ~332GB/s DMA ceiling; the first two units
                # fan out across 4 rings so the DMA engines saturate
                # immediately instead of ramping.
                for p in range(UW // 512):
                    ps = slice(p * 512, (p + 1) * 512)
                    gs = slice(c0 + p * 512, c0 + (p + 1) * 512)
                    nc.sync.dma_start(out=x0[:, ps], in_=xc[0:128, gs])
                    nc.sync.dma_start(out=x1[:, ps], in_=xc[128:256, gs])
                    nc.scalar.dma_start(out=t0[:, ps], in_=xt[0:128, gs])
                    nc.scalar.dma_start(out=t1[:, ps], in_=xt[128:256, gs])
                units[u] = (x0, x1, t0, t1)

            p1state = [None] * NU
            p2state = [None] * NU

            def emit_p1_f(u, f):
                x0, x1, t0, t1 = units[u]
                if f == 0:
                    p1state[u] = pcdp.tile([63, UW], F32, tag="pcd", name=f"pcd{u}")
                pcd = p1state[u]
                fsl = slice(f * 512, (f + 1) * 512)
                nc.tensor.matmul(pcd[0:R, fsl], w2sb[:, 0:R], x0[:, fsl],
                                 start=True, stop=False)
                nc.tensor.matmul(pcd[0:R, fsl], w2sb[:, R:2 * R],
                                 x1[:, fsl], start=False, stop=True)
                nc.tensor.matmul(pcd[32:63, fsl], w2sb[:, 0:R], t0[:, fsl],
                                 start=True, stop=False)
                nc.tensor.matmul(pcd[32:63, fsl], w2sb[:, R:2 * R],
                                 t1[:, fsl], start=False, stop=True)

            def emit_copies(u):
                pcd = p1state[u]
                LG, RG = LGs[u % 2], RGs[u % 2]
                # one wide copy per operand: C on ACT, D on DVE
                nc.scalar.copy(RG[0:R, :], pcd[0:R, :])
                nc.vector.tensor_copy(LG[32:63, :], pcd[32:63, :])

            def emit_p2_half(u, j):
                LG, RG = LGs[u % 2], RGs[u % 2]
                c0 = u * UW
                if j == 0:
                    p2state[u] = (outp.tile([128, UW], F16, tag="o0", name=f"o0_{u}"),
                                  outp.tile([128, UW], F16, tag="o1", name=f"o1_{u}"))
                oj = p2state[u][j]
                for pr in range(2):          # channel pairs
                    p2 = p2p.tile([128, 2 * N], F32, tag="p2")
                    for s in range(2):
                        il = 2 * pr + s
                        csl = slice(il * N, (il + 1) * N)
                        jsl = slice(il * N + j * 128, il * N + (j + 1) * 128)
                        nc.tensor.matmul(p2[:, s * N:(s + 1) * N],
                                         LG[:, jsl], RG[:, csl],
                                         start=True, stop=True)
                    osl = slice(2 * pr * N, (2 * pr + 2) * N)
                    # split casts across ACT/DVE to balance engine load
                    if (j + pr) % 2 == 0:
                        nc.vector.tensor_copy(oj[:, osl], p2[:])
                    else:
                        nc.scalar.copy(oj[:, osl], p2[:])
                # outputs drain on the GpSimd ring as produced (inputs
                # keep the SP/ACT rings), in 512-col pieces. The last two
                # units ride the SP/ACT rings instead - input traffic is
                # done by then and GpSimd's expensive dge drain overlaps
                # the remaining compute instead of extending the tail.
                ring = (nc.gpsimd if u < NU - 2
                        else (nc.sync if j == 0 else nc.scalar))
                for p in range(UW // 512):
                    ps = slice(p * 512, (p + 1) * 512)
                    gs = slice(c0 + p * 512, c0 + (p + 1) * 512)
                    ring.dma_start(out=out[j * 128:(j + 1) * 128, gs],
                                   in_=oj[:, ps])

            # software pipeline, interleaved at f-block granularity so the
            # PE queue always has ready P2 work from unit u-1 between P1
            # groups of unit u that may stall on input DMA.
            emit_dma(0)
            emit_dma(1)
            emit_p1_f(0, 0)
            emit_p1_f(0, 1)
            emit_copies(0)
            for u in range(1, NU):
                if u + 1 < NU:
                    emit_dma(u + 1)
                emit_p1_f(u, 0)
                emit_p2_half(u - 1, 0)
                emit_p1_f(u, 1)
                emit_p2_half(u - 1, 1)
                emit_copies(u)
            emit_p2_half(NU - 1, 0)
            emit_p2_half(NU - 1, 1)

    nc.finalize()
    return nc


_NC = None


def kernel(x: np.ndarray) -> np.ndarray:
    global _NC, LAST_RESULTS
    x = np.asarray(x)
    assert x.shape == (B, M, N, I), x.shape

    W = _basis().astype(NPDT)          # [256, 31]
    Wt = W.T.copy()                    # [31, 256]
    w2_np = np.concatenate([W[0:128, :], W[128:256, :]], axis=1)  # [128, 62]
    wtile = np.tile(Wt, (1, UW // N))                             # [31, 1024]
    wz_np = np.concatenate([wtile, np.zeros((1, UW), NPDT)], axis=0)
    zw_np = np.concatenate([np.zeros((1, UW), NPDT), wtile], axis=0)

    if _NC is None:
        _NC = _build_nc()

    xq = np.asarray(x, dtype=NPDT)
    in_maps = []
    for b in range(B):
        xcm = np.ascontiguousarray(xq[b].transpose(0, 2, 1)).reshape(M, FREE)
        xtm = np.ascontiguousarray(xq[b].transpose(1, 2, 0)).reshape(N, I * M)
        in_maps.append({
            "xc": xcm, "xt": xtm,
            "w2": w2_np, "wz": wz_np, "zw": zw_np,
        })

    trace = bool(int(os.environ.get("KERNEL_TRACE", "0")))
    if trace:
        _ensure_ntff_hook()
    last_err = None
    for attempt in range(3):
        try:
            LAST_RESULTS = run_bass_kernel_spmd(_NC, in_maps, list(range(B)),
                                                trace=trace and attempt == 0)
            break
        except Exception as e:  # rare transient NRT_EXEC_UNIT_UNRECOVERABLE
            last_err = e
            import time as _time
            _time.sleep(2.0)
            try:
                import jax
                jax.clear_caches()
                jax.extend.backend.clear_backends()
            except Exception:
                pass
    else:
        raise last_err

    out = np.empty((B, M, N, I), np.float32)
    for b in range(B):
        dev = LAST_RESULTS.results[b]["out"].astype(np.float32).reshape(M, I, N)
        out[b] = dev.transpose(0, 2, 1)
    return out



# revision 16
# speedup vs baseline: 1.2533x; 1.0435x over previous
"""Low-pass FFT filtering kernel for Trainium2 (8 NeuronCores).

Math: reference does, per (batch b, channel i), with X = x[b,:,:,i] (256x256):
    out_i = irfft(rfft(X, axis=0) * mask) + irfft(rfft(X, axis=1) * mask)
with mask keeping rfft modes 0..15 (ortho norm). That filter is an orthogonal
projection P = W @ W.T where W [256, 31] is the orthonormal basis
{1/sqrt(n), sqrt(2/n)cos(2pi k t/n), -sqrt(2/n)sin(2pi k t/n)}_{k=1..15}.
So  out_i = P @ X_i + X_i @ P = W @ (W.T @ X_i) + (X_i @ W) @ W.T.

Device schedule (per core = one batch, channel-major layouts):
  C = W.T @ Xcm   [31, I*N]   (Xcm = x[b] as [m, (i, n)])
  D = W.T @ Xt    [31, I*M]   (Xt  = x[b] as [n, (i, m)], host-transposed)
  out[m-tile, n'] per (i, j):  single K=63 matmul with
     lhsT = [Wt_j ; 0 ; D_i,j]  (63 x 128),  rhs = [C_i ; 0 ; Wt] (63 x 256)
  which accumulates both terms in one PSUM pass.
Inputs/weights are fp16 on device (PE runs fp16 at full rate vs 4x-cost
fp32 LOW_HIGH mode); accumulation is fp32 in PSUM; the output is staged fp16
on device and upcast to fp32 on host (rel err ~7e-4 end to end).
Sharding: batch b -> core b (8 cores, no communication).
"""

import os
import sys
import types

import numpy as np

import concourse.bass as bass
import concourse.bacc as bacc
import concourse.tile as tile
from concourse import mybir
from concourse.bass_utils import run_bass_kernel_spmd

B, M, N, I = 8, 256, 256, 32
KMAX = 16           # modes kept: 0..15
R = 2 * KMAX - 1    # 31 real basis vectors
FREE = I * N        # 8192
UW = 1024           # unit width (cols) = 4 channels
NU = FREE // UW     # 8 units
F32 = mybir.dt.float32
F16 = mybir.dt.float16
NPDT = np.float16

LAST_RESULTS = None  # BassKernelResults of the most recent run (for test.py)


def _ensure_ntff_hook():
    """Provide antenv.axon_hooks if the image lacks it, so trace=True works."""
    try:
        from antenv.axon_hooks import get_axon_ntff_profile_hook  # noqa: F401
        return
    except ImportError:
        pass
    try:
        from trn_agent_boot.trn_boot import _ntff_profile_via_ctypes
        hook = _ntff_profile_via_ctypes("/opt/axon/libaxon_pjrt.so")
    except Exception:
        hook = None
    mod = types.ModuleType("antenv.axon_hooks")
    _state = {"hook": hook}
    mod.get_axon_ntff_profile_hook = lambda: _state["hook"]
    mod.set_axon_ntff_profile_hook = lambda h: _state.update(hook=h)
    sys.modules["antenv.axon_hooks"] = mod
    try:
        import antenv
        antenv.axon_hooks = mod
    except ImportError:
        pass


def _basis():
    t = np.arange(N)
    cols = [np.ones(N) / np.sqrt(N)]
    for k in range(1, KMAX):
        cols.append(np.sqrt(2.0 / N) * np.cos(2 * np.pi * k * t / N))
        cols.append(-np.sqrt(2.0 / N) * np.sin(2 * np.pi * k * t / N))
    return np.stack(cols, axis=1).astype(np.float32)  # [256, 31]


def _build_nc():
    nc = bacc.Bacc("TRN2", target_bir_lowering=False, debug=False,
                   enable_asserts=False, num_devices=8)

    xc = nc.declare_dram_parameter("xc", [M, FREE], F16, isOutput=False)
    xt = nc.declare_dram_parameter("xt", [N, I * M], F16, isOutput=False)
    w2 = nc.declare_dram_parameter("w2", [128, 2 * R], F16, isOutput=False)
    wz = nc.declare_dram_parameter("wz", [R + 1, UW], F16, isOutput=False)
    zw = nc.declare_dram_parameter("zw", [R + 1, UW], F16, isOutput=False)
    out = nc.declare_dram_parameter("out", [M, FREE], F16, isOutput=True)

    with tile.TileContext(nc) as tc:
        with (
            tc.tile_pool(name="const", bufs=1) as constp,
            tc.tile_pool(name="xin", bufs=3) as xin,
            tc.tile_pool(name="oput", bufs=2) as outp,
            tc.tile_pool(name="pcd", bufs=2, space=bass.MemorySpace.PSUM) as pcdp,
            tc.tile_pool(name="p2", bufs=4, space=bass.MemorySpace.PSUM) as p2p,
        ):
            w2sb = constp.tile([128, 2 * R], F16)
            nc.sync.dma_start(out=w2sb[:], in_=w2[:])

            # persistent ping-pong staging tiles; const rows loaded ONCE:
            #   LG rows 0..30 = tiled W^T (lhsT const), row 31 = 0,
            #      rows 32..62 <- D per unit
            #   RG rows 0..30 <- C per unit, row 31 = 0,
            #      rows 32..62 = tiled W^T (rhs const)
            LGs = [constp.tile([63, UW], F16, tag=f"lg{k}", name=f"lg{k}")
                   for k in range(2)]
            RGs = [constp.tile([63, UW], F16, tag=f"rg{k}", name=f"rg{k}")
                   for k in range(2)]
            for k in range(2):
                nc.gpsimd.dma_start(out=LGs[k][0:32, :], in_=wz[:])
                nc.gpsimd.dma_start(out=RGs[k][31:63, :], in_=zw[:])

            units = [None] * NU

            def emit_dma(u):
                c0 = u * UW
                x0 = xin.tile([128, UW], F16, tag="x0")
                x1 = xin.tile([128, UW], F16, tag="x1")
                t0 = xin.tile([128, UW], F16, tag="t0")
                t1 = xin.tile([128, UW], F16, tag="t1")
                # 512-col pieces so the first matmuls start after only 128KB
                # has landed and per-piece deps release early. Steady state
                # uses 2 rings (x on SP, xt on ACT) whose combined descriptor
                # rate covers the ~332GB/s DMA ceiling; the first two units
                # fan out across 4 rings so the DMA engines saturate
                # immediately instead of ramping.
                for p in range(UW // 512):
                    ps = slice(p * 512, (p + 1) * 512)
                    gs = slice(c0 + p * 512, c0 + (p + 1) * 512)
                    nc.sync.dma_start(out=x0[:, ps], in_=xc[0:128, gs])
                    nc.sync.dma_start(out=x1[:, ps], in_=xc[128:256, gs])
                    nc.scalar.dma_start(out=t0[:, ps], in_=xt[0:128, gs])
                    nc.scalar.dma_start(out=t1[:, ps], in_=xt[128:256, gs])
                units[u] = (x0, x1, t0, t1)

            p1state = [None] * NU
            p2state = [None] * NU

            def emit_p1_f(u, f):
                x0, x1, t0, t1 = units[u]
                if f == 0:
                    p1state[u] = pcdp.tile([63, UW], F32, tag="pcd", name=f"pcd{u}")
                pcd = p1state[u]
                fsl = slice(f * 512, (f + 1) * 512)
                nc.tensor.matmul(pcd[0:R, fsl], w2sb[:, 0:R], x0[:, fsl],
                                 start=True, stop=False)
                nc.tensor.matmul(pcd[0:R, fsl], w2sb[:, R:2 * R],
                                 x1[:, fsl], start=False, stop=True)
                nc.tensor.matmul(pcd[32:63, fsl], w2sb[:, 0:R], t0[:, fsl],
                                 start=True, stop=False)
                nc.tensor.matmul(pcd[32:63, fsl], w2sb[:, R:2 * R],
                                 t1[:, fsl], start=False, stop=True)

            def emit_copies(u):
                pcd = p1state[u]
                LG, RG = LGs[u % 2], RGs[u % 2]
                # both copies on DVE: the ACT sequencer carries the xt DMA
                # ring, and compute on it would delay t-piece descriptor
                # generation (engine-op + DIRECT2D serialize per sequencer)
                nc.vector.tensor_copy(RG[0:R, :], pcd[0:R, :])
                nc.vector.tensor_copy(LG[32:63, :], pcd[32:63, :])

            def emit_p2_half(u, j):
                LG, RG = LGs[u % 2], RGs[u % 2]
                c0 = u * UW
                if j == 0:
                    p2state[u] = (outp.tile([128, UW], F16, tag="o0", name=f"o0_{u}"),
                                  outp.tile([128, UW], F16, tag="o1", name=f"o1_{u}"))
                oj = p2state[u][j]
                for pr in range(2):          # channel pairs
                    p2 = p2p.tile([128, 2 * N], F32, tag="p2")
                    for s in range(2):
                        il = 2 * pr + s
                        csl = slice(il * N, (il + 1) * N)
                        jsl = slice(il * N + j * 128, il * N + (j + 1) * 128)
                        nc.tensor.matmul(p2[:, s * N:(s + 1) * N],
                                         LG[:, jsl], RG[:, csl],
                                         start=True, stop=True)
                    osl = slice(2 * pr * N, (2 * pr + 2) * N)
                    # split casts across ACT/DVE to balance engine load
                    if (j + pr) % 2 == 0:
                        nc.vector.tensor_copy(oj[:, osl], p2[:])
                    else:
                        nc.scalar.copy(oj[:, osl], p2[:])
                # outputs drain on the GpSimd ring as produced (inputs
                # keep the SP/ACT rings), in 512-col pieces. The last two
                # units ride the SP/ACT rings instead - input traffic is
                # done by then and GpSimd's expensive dge drain overlaps
                # the remaining compute instead of extending the tail.
                ring = nc.gpsimd if u < NU - 1 else nc.sync
                for p in range(UW // 512):
                    ps = slice(p * 512, (p + 1) * 512)
                    gs = slice(c0 + p * 512, c0 + (p + 1) * 512)
                    ring.dma_start(out=out[j * 128:(j + 1) * 128, gs],
                                   in_=oj[:, ps])

            # software pipeline, interleaved at f-block granularity so the
            # PE queue always has ready P2 work from unit u-1 between P1
            # groups of unit u that may stall on input DMA.
            emit_dma(0)
            emit_dma(1)
            emit_p1_f(0, 0)
            emit_p1_f(0, 1)
            emit_copies(0)
            for u in range(1, NU):
                if u + 1 < NU:
                    emit_dma(u + 1)
                emit_p1_f(u, 0)
                emit_p2_half(u - 1, 0)
                emit_p1_f(u, 1)
                emit_p2_half(u - 1, 1)
                emit_copies(u)
            emit_p2_half(NU - 1, 0)
            emit_p2_half(NU - 1, 1)

    nc.finalize()
    return nc


_NC = None


def kernel(x: np.ndarray) -> np.ndarray:
    global _NC, LAST_RESULTS
    x = np.asarray(x)
    assert x.shape == (B, M, N, I), x.shape

    W = _basis().astype(NPDT)          # [256, 31]
    Wt = W.T.copy()                    # [31, 256]
    w2_np = np.concatenate([W[0:128, :], W[128:256, :]], axis=1)  # [128, 62]
    wtile = np.tile(Wt, (1, UW // N))                             # [31, 1024]
    wz_np = np.concatenate([wtile, np.zeros((1, UW), NPDT)], axis=0)
    zw_np = np.concatenate([np.zeros((1, UW), NPDT), wtile], axis=0)

    if _NC is None:
        _NC = _build_nc()

    xq = np.asarray(x, dtype=NPDT)
    in_maps = []
    for b in range(B):
        xcm = np.ascontiguousarray(xq[b].transpose(0, 2, 1)).reshape(M, FREE)
        xtm = np.ascontiguousarray(xq[b].transpose(1, 2, 0)).reshape(N, I * M)
        in_maps.append({
            "xc": xcm, "xt": xtm,
            "w2": w2_np, "wz": wz_np, "zw": zw_np,
        })

    trace = bool(int(os.environ.get("KERNEL_TRACE", "0")))
    if trace:
        _ensure_ntff_hook()
    last_err = None
    for attempt in range(3):
        try:
            LAST_RESULTS = run_bass_kernel_spmd(_NC, in_maps, list(range(B)),
                                                trace=trace and attempt == 0)
            break
        except Exception as e:  # rare transient NRT_EXEC_UNIT_UNRECOVERABLE
            last_err = e
            import time as _time
            _time.sleep(2.0)
            try:
                import jax
                jax.clear_caches()
                jax.extend.backend.clear_backends()
            except Exception:
                pass
    else:
        raise last_err

    out = np.empty((B, M, N, I), np.float32)
    for b in range(B):
        dev = LAST_RESULTS.results[b]["out"].astype(np.float32).reshape(M, I, N)
        out[b] = dev.transpose(0, 2, 1)
    return out



# revision 17
# speedup vs baseline: 1.2637x; 1.0083x over previous
"""Low-pass FFT filtering kernel for Trainium2 (8 NeuronCores).

Math: reference does, per (batch b, channel i), with X = x[b,:,:,i] (256x256):
    out_i = irfft(rfft(X, axis=0) * mask) + irfft(rfft(X, axis=1) * mask)
with mask keeping rfft modes 0..15 (ortho norm). That filter is an orthogonal
projection P = W @ W.T where W [256, 31] is the orthonormal basis
{1/sqrt(n), sqrt(2/n)cos(2pi k t/n), -sqrt(2/n)sin(2pi k t/n)}_{k=1..15}.
So  out_i = P @ X_i + X_i @ P = W @ (W.T @ X_i) + (X_i @ W) @ W.T.

Device schedule (per core = one batch, channel-major layouts):
  C = W.T @ Xcm   [31, I*N]   (Xcm = x[b] as [m, (i, n)])
  D = W.T @ Xt    [31, I*M]   (Xt  = x[b] as [n, (i, m)], host-transposed)
  out[m-tile, n'] per (i, j):  single K=63 matmul with
     lhsT = [Wt_j ; 0 ; D_i,j]  (63 x 128),  rhs = [C_i ; 0 ; Wt] (63 x 256)
  which accumulates both terms in one PSUM pass.
Inputs/weights are fp16 on device (PE runs fp16 at full rate vs 4x-cost
fp32 LOW_HIGH mode); accumulation is fp32 in PSUM; the output is staged fp16
on device and upcast to fp32 on host (rel err ~7e-4 end to end).
Sharding: batch b -> core b (8 cores, no communication).
"""

import os
import sys
import types

import numpy as np

import concourse.bass as bass
import concourse.bacc as bacc
import concourse.tile as tile
from concourse import mybir
from concourse.bass_utils import run_bass_kernel_spmd

B, M, N, I = 8, 256, 256, 32
KMAX = 16           # modes kept: 0..15
R = 2 * KMAX - 1    # 31 real basis vectors
FREE = I * N        # 8192
UW = 1024           # unit width (cols) = 4 channels
NU = FREE // UW     # 8 units
F32 = mybir.dt.float32
F16 = mybir.dt.float16
NPDT = np.float16

LAST_RESULTS = None  # BassKernelResults of the most recent run (for test.py)


def _ensure_ntff_hook():
    """Provide antenv.axon_hooks if the image lacks it, so trace=True works."""
    try:
        from antenv.axon_hooks import get_axon_ntff_profile_hook  # noqa: F401
        return
    except ImportError:
        pass
    try:
        from trn_agent_boot.trn_boot import _ntff_profile_via_ctypes
        hook = _ntff_profile_via_ctypes("/opt/axon/libaxon_pjrt.so")
    except Exception:
        hook = None
    mod = types.ModuleType("antenv.axon_hooks")
    _state = {"hook": hook}
    mod.get_axon_ntff_profile_hook = lambda: _state["hook"]
    mod.set_axon_ntff_profile_hook = lambda h: _state.update(hook=h)
    sys.modules["antenv.axon_hooks"] = mod
    try:
        import antenv
        antenv.axon_hooks = mod
    except ImportError:
        pass


def _basis():
    t = np.arange(N)
    cols = [np.ones(N) / np.sqrt(N)]
    for k in range(1, KMAX):
        cols.append(np.sqrt(2.0 / N) * np.cos(2 * np.pi * k * t / N))
        cols.append(-np.sqrt(2.0 / N) * np.sin(2 * np.pi * k * t / N))
    return np.stack(cols, axis=1).astype(np.float32)  # [256, 31]


def _build_nc():
    nc = bacc.Bacc("TRN2", target_bir_lowering=False, debug=False,
                   enable_asserts=False, num_devices=8)

    xc = nc.declare_dram_parameter("xc", [M, FREE], F16, isOutput=False)
    xt = nc.declare_dram_parameter("xt", [N, I * M], F16, isOutput=False)
    w2 = nc.declare_dram_parameter("w2", [128, 2 * R], F16, isOutput=False)
    wz = nc.declare_dram_parameter("wz", [R + 1, UW], F16, isOutput=False)
    zw = nc.declare_dram_parameter("zw", [R + 1, UW], F16, isOutput=False)
    out = nc.declare_dram_parameter("out", [M, FREE], F16, isOutput=True)

    with tile.TileContext(nc) as tc:
        with (
            tc.tile_pool(name="const", bufs=1) as constp,
            tc.tile_pool(name="xin", bufs=3) as xin,
            tc.tile_pool(name="oput", bufs=2) as outp,
            tc.tile_pool(name="pcd", bufs=2, space=bass.MemorySpace.PSUM) as pcdp,
            tc.tile_pool(name="p2", bufs=4, space=bass.MemorySpace.PSUM) as p2p,
        ):
            w2sb = constp.tile([128, 2 * R], F16)
            nc.sync.dma_start(out=w2sb[:], in_=w2[:])

            # persistent ping-pong staging tiles; const rows loaded ONCE:
            #   LG rows 0..30 = tiled W^T (lhsT const), row 31 = 0,
            #      rows 32..62 <- D per unit
            #   RG rows 0..30 <- C per unit, row 31 = 0,
            #      rows 32..62 = tiled W^T (rhs const)
            LGs = [constp.tile([63, UW], F16, tag=f"lg{k}", name=f"lg{k}")
                   for k in range(2)]
            RGs = [constp.tile([63, UW], F16, tag=f"rg{k}", name=f"rg{k}")
                   for k in range(2)]
            for k in range(2):
                nc.gpsimd.dma_start(out=LGs[k][0:32, :], in_=wz[:])
                nc.gpsimd.dma_start(out=RGs[k][31:63, :], in_=zw[:])

            units = [None] * NU

            def emit_dma(u):
                c0 = u * UW
                x0 = xin.tile([128, UW], F16, tag="x0")
                x1 = xin.tile([128, UW], F16, tag="x1")
                t0 = xin.tile([128, UW], F16, tag="t0")
                t1 = xin.tile([128, UW], F16, tag="t1")
                # 512-col pieces so the first matmuls start after only 128KB
                # has landed and per-piece deps release early. Steady state
                # uses 2 rings (x on SP, xt on ACT) whose combined descriptor
                # rate covers the ~332GB/s DMA ceiling; the first two units
                # fan out across 4 rings so the DMA engines saturate
                # immediately instead of ramping.
                for p in range(UW // 512):
                    ps = slice(p * 512, (p + 1) * 512)
                    gs = slice(c0 + p * 512, c0 + (p + 1) * 512)
                    nc.sync.dma_start(out=x0[:, ps], in_=xc[0:128, gs])
                    nc.sync.dma_start(out=x1[:, ps], in_=xc[128:256, gs])
                    nc.scalar.dma_start(out=t0[:, ps], in_=xt[0:128, gs])
                    nc.scalar.dma_start(out=t1[:, ps], in_=xt[128:256, gs])
                units[u] = (x0, x1, t0, t1)

            p1state = [None] * NU
            p2state = [None] * NU

            def emit_p1_f(u, f):
                x0, x1, t0, t1 = units[u]
                if f == 0:
                    p1state[u] = pcdp.tile([63, UW], F32, tag="pcd", name=f"pcd{u}")
                pcd = p1state[u]
                fsl = slice(f * 512, (f + 1) * 512)
                nc.tensor.matmul(pcd[0:R, fsl], w2sb[:, 0:R], x0[:, fsl],
                                 start=True, stop=False)
                nc.tensor.matmul(pcd[0:R, fsl], w2sb[:, R:2 * R],
                                 x1[:, fsl], start=False, stop=True)
                nc.tensor.matmul(pcd[32:63, fsl], w2sb[:, 0:R], t0[:, fsl],
                                 start=True, stop=False)
                nc.tensor.matmul(pcd[32:63, fsl], w2sb[:, R:2 * R],
                                 t1[:, fsl], start=False, stop=True)

            def emit_copies(u):
                pcd = p1state[u]
                LG, RG = LGs[u % 2], RGs[u % 2]
                # both copies on DVE: the ACT sequencer carries the xt DMA
                # ring, and compute on it would delay t-piece descriptor
                # generation (engine-op + DIRECT2D serialize per sequencer)
                nc.vector.tensor_copy(RG[0:R, :], pcd[0:R, :])
                nc.vector.tensor_copy(LG[32:63, :], pcd[32:63, :])

            def emit_p2_half(u, j):
                LG, RG = LGs[u % 2], RGs[u % 2]
                c0 = u * UW
                if j == 0:
                    p2state[u] = (outp.tile([128, UW], F16, tag="o0", name=f"o0_{u}"),
                                  outp.tile([128, UW], F16, tag="o1", name=f"o1_{u}"))
                oj = p2state[u][j]
                for pr in range(2):          # channel pairs
                    p2 = p2p.tile([128, 2 * N], F32, tag="p2")
                    for s in range(2):
                        il = 2 * pr + s
                        csl = slice(il * N, (il + 1) * N)
                        jsl = slice(il * N + j * 128, il * N + (j + 1) * 128)
                        nc.tensor.matmul(p2[:, s * N:(s + 1) * N],
                                         LG[:, jsl], RG[:, csl],
                                         start=True, stop=True)
                    osl = slice(2 * pr * N, (2 * pr + 2) * N)
                    nc.vector.tensor_copy(oj[:, osl], p2[:])
                # outputs drain on the GpSimd ring as produced (inputs
                # keep the SP/ACT rings), in 512-col pieces. The last two
                # units ride the SP/ACT rings instead - input traffic is
                # done by then and GpSimd's expensive dge drain overlaps
                # the remaining compute instead of extending the tail.
                ring = nc.gpsimd if u < NU - 1 else nc.sync
                for p in range(UW // 512):
                    ps = slice(p * 512, (p + 1) * 512)
                    gs = slice(c0 + p * 512, c0 + (p + 1) * 512)
                    ring.dma_start(out=out[j * 128:(j + 1) * 128, gs],
                                   in_=oj[:, ps])

            # software pipeline, interleaved at f-block granularity so the
            # PE queue always has ready P2 work from unit u-1 between P1
            # groups of unit u that may stall on input DMA.
            emit_dma(0)
            emit_dma(1)
            emit_p1_f(0, 0)
            emit_p1_f(0, 1)
            emit_copies(0)
            for u in range(1, NU):
                if u + 1 < NU:
                    emit_dma(u + 1)
                emit_p1_f(u, 0)
                emit_p2_half(u - 1, 0)
                emit_p1_f(u, 1)
                emit_p2_half(u - 1, 1)
                emit_copies(u)
            emit_p2_half(NU - 1, 0)
            emit_p2_half(NU - 1, 1)

    nc.finalize()
    return nc


_NC = None


def kernel(x: np.ndarray) -> np.ndarray:
    global _NC, LAST_RESULTS
    x = np.asarray(x)
    assert x.shape == (B, M, N, I), x.shape

    W = _basis().astype(NPDT)          # [256, 31]
    Wt = W.T.copy()                    # [31, 256]
    w2_np = np.concatenate([W[0:128, :], W[128:256, :]], axis=1)  # [128, 62]
    wtile = np.tile(Wt, (1, UW // N))                             # [31, 1024]
    wz_np = np.concatenate([wtile, np.zeros((1, UW), NPDT)], axis=0)
    zw_np = np.concatenate([np.zeros((1, UW), NPDT), wtile], axis=0)

    if _NC is None:
        _NC = _build_nc()

    xq = np.asarray(x, dtype=NPDT)
    in_maps = []
    for b in range(B):
        xcm = np.ascontiguousarray(xq[b].transpose(0, 2, 1)).reshape(M, FREE)
        xtm = np.ascontiguousarray(xq[b].transpose(1, 2, 0)).reshape(N, I * M)
        in_maps.append({
            "xc": xcm, "xt": xtm,
            "w2": w2_np, "wz": wz_np, "zw": zw_np,
        })

    trace = bool(int(os.environ.get("KERNEL_TRACE", "0")))
    if trace:
        _ensure_ntff_hook()
    last_err = None
    for attempt in range(3):
        try:
            LAST_RESULTS = run_bass_kernel_spmd(_NC, in_maps, list(range(B)),
                                                trace=trace and attempt == 0)
            break
        except Exception as e:  # rare transient NRT_EXEC_UNIT_UNRECOVERABLE
            last_err = e
            import time as _time
            _time.sleep(2.0)
            try:
                import jax
                jax.clear_caches()
                jax.extend.backend.clear_backends()
            except Exception:
                pass
    else:
        raise last_err

    out = np.empty((B, M, N, I), np.float32)
    for b in range(B):
        dev = LAST_RESULTS.results[b]["out"].astype(np.float32).reshape(M, I, N)
        out[b] = dev.transpose(0, 2, 1)
    return out

